# revision 35
# baseline (speedup 1.0000x reference)
"""Banded (sparse) attention + projections on 8 Trainium2 NeuronCores.

Problem: nn_Attention_old_90211493085279
  x [2, 2048, 1024] -> qkv = x @ Wqkv, banded softmax(QK^T) V (half-width 8),
  out = attn @ Wproj + bproj.

Sharding: (batch x tokens) across the 8 cores -- each core owns a contiguous
block of 512 token rows. The attention band is 17 wide, so each core needs an
8-token halo of K/V context only: NO collectives.

v4 design:
  - inputs packed host-side into a few large DMAs spread over the
    sync/scalar/gpsimd issue queues (a dma_start costs ~600 ns of issue time
    on its queue engine); xh + the first qk weight block go first so real
    matmuls can start ~15 us in, behind a PE warmup chain that releases the
    HAM clock-gate (2.4 GHz) before real work arrives.
  - attention runs on 112-row query tiles whose k/v window is EXACTLY 128
    (112 + 2*8), so each (head, tile) needs ONE score matmul and ONE AV
    matmul -- no separate 16-row band remainder strip.
  - AV matmul runs in NATURAL layout (attention-weight strip stationary, V
    moving), so the softmax denominator lands per-partition: normalization
    is a native reciprocal [128,5] + tensor_scalar_mul. A [1,512]
    single-partition reciprocal would cost 4 us on DVE; this costs ~0.1 us.
  - normalized O tiles are transposed to the [dims, tokens] layout the
    output projection needs via XBAR DMA-transposes on the otherwise idle
    DMA engines, alternating sync/scalar issue queues.
  - output projection is c-outer across 8 PSUM banks (reusing every
    attention pool's bank), so only the last 8-matmul column depends on the
    final head; bias is folded in as a K=1 matmul against a ones row; the
    PSUM->SBUF copies and output DMAs alternate scalar/vector engines and
    scalar/sync queues.
"""

import sys

sys.path.insert(0, "/opt/trn_rl_repo")

import ml_dtypes
import numpy as np

import concourse.bass as bass
import concourse.tile as tile
from concourse import bacc, mybir
from concourse.bass_utils import run_bass_kernel_spmd

F32 = mybir.dt.float32
BF16 = mybir.dt.bfloat16
AF = mybir.ActivationFunctionType

B, N, C, H, HD, W = 2, 2048, 1024, 16, 64, 8
SCALE = float(HD) ** -0.5
CORES = 8
TOK = 512            # token rows owned per core
HALO = TOK + 2 * W   # 528 k/v context tokens per core
QT = 112             # query rows per attention tile (window = QT+2W = 128)
NWT = 5              # attention tiles per core: 4x112 + 1x64
NWARM = 40           # PE warmup matmuls (N=256 each)

# per-tile (query-rows, window-rows); last tile is the 64-row remainder
WTS = [(112, 128), (112, 128), (112, 128), (112, 128), (64, 80)]

_CACHE = {}


def _build_nc(dbg=False):
    nc = bacc.Bacc(None, target_bir_lowering=False)
    xhp = nc.dram_tensor("xhp", [128, 8 * HALO], BF16, kind="ExternalInput")
    wvp = nc.dram_tensor("wvp", [128, 8192], BF16, kind="ExternalInput")
    wqkp = nc.dram_tensor("wqkp", [128, 16384], BF16, kind="ExternalInput")
    wpp = nc.dram_tensor("wpp", [128, 8192], BF16, kind="ExternalInput")
    mA = nc.dram_tensor("mA", [128, TOK], BF16, kind="ExternalInput")
    bT = nc.dram_tensor("bT", [1, C], BF16, kind="ExternalInput")
    outT = nc.dram_tensor("outT", [C, TOK], BF16, kind="ExternalOutput")

    with tile.TileContext(nc) as tc:
        with tc.tile_pool(name="persist", bufs=1) as pp:
            # ---- persistent SBUF ----
            wmA = pp.tile([128, 128], BF16, tag="wmA", name="wmA")
            wmB = pp.tile([128, 256], BF16, tag="wmB", name="wmB")
            ones_r = pp.tile([1, TOK], BF16, tag="ones_r", name="ones_r")
            xh = pp.tile([128, 8 * HALO], BF16, tag="xh", name="xh")
            wv_sb = pp.tile([128, 8192], BF16, tag="wv", name="wv")
            wqk_sb = pp.tile([128, 16384], BF16, tag="wqk", name="wqk")
            wp_sb = pp.tile([128, 8192], BF16, tag="wp", name="wp")
            mask_a = pp.tile([128, TOK], BF16, tag="mask_a", name="mask_a")
            biasT = pp.tile([1, C], BF16, tag="biasT", name="biasT")
            v1 = [pp.tile([pw, H, HD + 1], BF16, tag=f"v1_{t}", name=f"v1_{t}")
                  for t, (pq, pw) in enumerate(WTS)]
            # k-side qkT padded to 576 cols (zeros) so every score tile can
            # use a full 128-wide window matmul
            qkT = [pp.tile([128, 576 if m >= 8 else HALO], BF16,
                           tag=f"qkT{m}", name=f"qkT{m}") for m in range(16)]
            otn = [pp.tile([128, TOK], BF16, tag=f"otn{m}", name=f"otn{m}")
                   for m in range(8)]

            # warmup sources: first DVE work, no DMA dependency
            nc.vector.memset(wmA[:], 0.0)
            nc.vector.memset(wmB[:], 0.0)
            nc.vector.memset(ones_r[:], 1.0)
            for t in range(NWT):
                # ones column of v1 (disjoint from the v-proj copy columns)
                nc.vector.memset(v1[t][:, :, HD], 1.0)
            for m in range(8, 16):
                nc.vector.memset(qkT[m][:, HALO:576], 0.0)

            # ---- input DMAs: few, large, spread across issue queues ----
            # sync queue: the critical pair first (xh + first qk block),
            # fine-grained so the first projection starts ASAP; the sync
            # queue's DGE starts ~3 us before the scalar queue's
            nc.sync.dma_start(out=xh[:, 0:2112], in_=xhp[:, 0:2112])
            nc.sync.dma_start(out=wqk_sb[:, 0:1024], in_=wqkp[:, 0:1024])
            nc.sync.dma_start(out=xh[:, 2112:4224], in_=xhp[:, 2112:4224])
            nc.sync.dma_start(out=wqk_sb[:, 1024:2048], in_=wqkp[:, 1024:2048])
            nc.sync.dma_start(out=wv_sb[:, 0:4096], in_=wvp[:, 0:4096])
            nc.sync.dma_start(out=wv_sb[:, 4096:8192], in_=wvp[:, 4096:8192])
            nc.sync.dma_start(out=wp_sb[:], in_=wpp[:])
            # scalar queue: remaining qk weight blocks (m-major packing)
            for fm in range(1, 8):
                nc.scalar.dma_start(out=wqk_sb[:, 2048 * fm:2048 * (fm + 1)],
                                    in_=wqkp[:, 2048 * fm:2048 * (fm + 1)])
            # gpsimd (software DGE): small tensors
            nc.gpsimd.dma_start(out=mask_a[:], in_=mA[:])
            nc.gpsimd.dma_start(out=biasT[:], in_=bT[:])

            # PSUM budget (8 banks of 2 KiB):
            #   pa(shared with pb) 1 + pk 1 + stA 2 + otb 2 + pv 2 = 8
            with tc.tile_pool(name="psQ", bufs=1, space="PSUM") as psQ, \
                 tc.tile_pool(name="psSA", bufs=2, space="PSUM") as psSA, \
                 tc.tile_pool(name="psO", bufs=2, space="PSUM") as psO, \
                 tc.tile_pool(name="psV", bufs=2, space="PSUM") as psV, \
                 tc.tile_pool(name="atpa", bufs=2) as atpa, \
                 tc.tile_pool(name="recp", bufs=2) as recp, \
                 tc.tile_pool(name="bcp", bufs=2) as bcp, \
                 tc.tile_pool(name="outp", bufs=8) as outp:

                # ---- PE warmup: serialized dummy matmuls keep HAM busy
                # while the input DMAs stream ----
                psw = psV.tile([128, 512], F32, tag="pv", name="warm")
                for i in range(NWARM):
                    nc.tensor.matmul(psw[:, 0:256], wmA[:], wmB[:],
                                     start=True, stop=True)
                wsink = atpa.tile([128, TOK], BF16, tag="atA", name="wsink")
                nc.vector.tensor_copy(wsink[:, 0:256], psw[:, 0:256])

                ats = {}

                def emit_qkproj(fm):
                    # q chunk m=fm (own tokens), k chunk m=8+fm (full halo)
                    pa = psQ.tile([128, 512], F32, tag="pa", name="pa")
                    for c in range(8):
                        nc.tensor.matmul(
                            pa[:],
                            wqk_sb[:, 2048 * fm + 128 * c:2048 * fm + 128 * (c + 1)],
                            xh[:, 528 * c + W:528 * c + W + TOK],
                            start=(c == 0), stop=(c == 7))
                    nc.vector.tensor_copy(qkT[fm][:, W:W + TOK], pa[:])
                    pk = psQ.tile([128, 512], F32, tag="pk", name="pk")
                    for c in range(8):
                        nc.tensor.matmul(
                            pk[:],
                            wqk_sb[:, 2048 * fm + 1024 + 128 * c:
                                   2048 * fm + 1024 + 128 * (c + 1)],
                            xh[:, 528 * c:528 * c + 512],
                            start=(c == 0), stop=(c == 7))
                    nc.scalar.copy(qkT[8 + fm][:, 0:512], pk[:])
                    pb = psQ.tile([128, 512], F32, tag="pa", name="pb")
                    for c in range(8):
                        nc.tensor.matmul(
                            pb[:, 0:2 * W],
                            wqk_sb[:, 2048 * fm + 1024 + 128 * c:
                                   2048 * fm + 1024 + 128 * (c + 1)],
                            xh[:, 528 * c + 512:528 * c + 528],
                            start=(c == 0), stop=(c == 7))
                    nc.scalar.copy(qkT[8 + fm][:, 512:528], pb[:, 0:2 * W])

                def emit_scores(fm, h):
                    # transposed score strips + exp + band mask, one head.
                    # tile i: window = halo[112i : 112i+128], queries
                    # 112i..112i+111 at strip cols 112i.. -- one matmul each.
                    off = (h % 2) * 64
                    stA = psSA.tile([128, TOK], F32, tag="stA", name="stA")
                    col = 0
                    for t, (pq, pw) in enumerate(WTS):
                        s = QT * t
                        q_ap = qkT[fm][off:off + 64, W + s:W + s + pq]
                        k1 = qkT[8 + fm][off:off + 64, s:s + 128]
                        nc.tensor.matmul(stA[:, col:col + pq], k1, q_ap,
                                         start=True, stop=True)
                        col += pq
                    atA = atpa.tile([128, TOK], BF16, tag="atA", name="atA")
                    nc.scalar.activation(atA[:], stA[:], AF.Exp)
                    nc.vector.tensor_mul(atA[:], atA[:], mask_a[:])
                    ats[h] = atA

                def emit_av(fm, h):
                    # transposed O strip [65, 512] per head (V stationary,
                    # attention weights moving); row 64 = softmax denominator.
                    # Normalization chain never touches the PE: DVE
                    # approx-reciprocal [1,512] -> GpSimd partition broadcast
                    # -> fused DVE multiply into otn.
                    off = (h % 2) * 64
                    atA = ats.pop(h)
                    otb = psO.tile([128, 512], F32, tag="otb", name="otb")
                    col = 0
                    for t, (pq, pw) in enumerate(WTS):
                        nc.tensor.matmul(otb[0:HD + 1, col:col + pq],
                                         v1[t][:, h, :],
                                         atA[0:pw, col:col + pq],
                                         start=True, stop=True)
                        col += pq
                    den = recp.tile([1, TOK], F32, tag="den", name="den")
                    nc.scalar.copy(den[:], otb[HD:HD + 1, :])
                    rec = recp.tile([1, TOK], F32, tag="rec", name="rec")
                    nc.vector.reciprocal_approx_fast(rec[:], den[:])
                    bc = bcp.tile([HD, TOK], F32, tag="bc", name="bc")
                    nc.gpsimd.partition_broadcast(bc[:], rec[0:1, :])
                    nc.vector.tensor_mul(otn[fm][off:off + 64, :],
                                         otb[0:HD, :], bc[:])

                def emit_vproj():
                    # v = x @ Wv in natural [token, head, dim+1] layout at
                    # the 112-stride window offsets (windows overlap; the
                    # matmul count is unchanged). 65th column = 1.0 so AV
                    # also sums the denominators. half0 groups first.
                    for half in range(2):
                        for t, (pq, pw) in enumerate(WTS):
                            s = QT * t
                            pv = psV.tile([128, 512], F32, tag="pv",
                                          name=f"pv{half}_{t}")
                            for c in range(8):
                                nc.tensor.matmul(
                                    pv[:pw, :],
                                    xh[:, 528 * c + s:528 * c + s + pw],
                                    wv_sb[:, 4096 * half + 512 * c:
                                          4096 * half + 512 * c + 512],
                                    start=(c == 0), stop=(c == 7))
                            nc.vector.tensor_copy(
                                v1[t][:, 8 * half:8 * half + 8, 0:HD],
                                pv[:pw, :].rearrange("p (h d) -> p h d", d=HD))

                # ---- emission order (engine queues are FIFO) ----
                emit_qkproj(0)
                emit_scores(0, 0)
                emit_scores(0, 1)
                emit_qkproj(1)
                emit_vproj()
                emit_scores(1, 2)
                emit_av(0, 0)
                emit_scores(1, 3)
                emit_av(0, 1)
                for fm in range(2, 8):
                    emit_qkproj(fm)
                    emit_av(fm - 1, 2 * fm - 2)
                    emit_scores(fm, 2 * fm)
                    emit_av(fm - 1, 2 * fm - 1)
                    emit_scores(fm, 2 * fm + 1)
                emit_av(7, 14)
                emit_av(7, 15)

                # ---- output projection, c-outer over 8 PSUM banks ----
                pf = [psV.tile([128, 512], F32, tag="pv", name="pf0"),
                      psV.tile([128, 512], F32, tag="pv", name="pf1"),
                      psQ.tile([128, 512], F32, tag="pa", name="pf2"),
                      psQ.tile([128, 512], F32, tag="pk", name="pf3"),
                      psSA.tile([128, 512], F32, tag="stA", name="pf4"),
                      psSA.tile([128, 512], F32, tag="stA", name="pf5"),
                      psO.tile([128, 512], F32, tag="otb", name="pf6"),
                      psO.tile([128, 512], F32, tag="otb", name="pf7")]
                for c in range(8):
                    for m in range(8):
                        nc.tensor.matmul(
                            pf[m][:],
                            wp_sb[:, 1024 * c + 128 * m:1024 * c + 128 * (m + 1)],
                            otn[c][:],
                            start=(c == 0), stop=False)
                for m in range(8):
                    # bias folded in as a K=1 matmul closing each group
                    nc.tensor.matmul(pf[m][:], biasT[0:1, 128 * m:128 * (m + 1)],
                                     ones_r[0:1, :], start=False, stop=True)
                for m in range(8):
                    ob = outp.tile([128, 512], BF16, tag="ob", name="ob")
                    if m % 2 == 0:
                        nc.scalar.copy(ob[:], pf[m][:])
                        nc.scalar.dma_start(out=outT[128 * m:128 * (m + 1), :],
                                            in_=ob[:])
                    else:
                        nc.vector.tensor_copy(ob[:], pf[m][:])
                        nc.sync.dma_start(out=outT[128 * m:128 * (m + 1), :],
                                          in_=ob[:])

    nc.finalize()
    return nc


def _get_nc(dbg=False):
    key = ("nc", dbg)
    if key not in _CACHE:
        _CACHE[key] = _build_nc(dbg)
    return _CACHE[key]


def _band_mask_np(n, w):
    i = np.arange(n)[:, None]
    j = np.arange(n)[None, :]
    lo = np.where(i <= w, 0, i - w)
    hi = np.where(n - i <= w, n - 1, i + w)
    return (j >= lo) & (j <= hi)


def _make_in_maps(x, Wqkv, Wproj, bproj):
    x = np.ascontiguousarray(np.asarray(x, dtype=np.float32))
    Wqkv = np.asarray(Wqkv, dtype=np.float32)
    Wproj = np.ascontiguousarray(np.asarray(Wproj, dtype=np.float32))
    bproj = np.asarray(bproj, dtype=np.float32)

    wqk_host = np.concatenate(
        [Wqkv[:, :C] * np.float32(SCALE), Wqkv[:, C:2 * C]], axis=1)
    wqk_host = np.ascontiguousarray(wqk_host).astype(ml_dtypes.bfloat16)
    wv_host = np.ascontiguousarray(Wqkv[:, 2 * C:]).astype(ml_dtypes.bfloat16)
    wp_host = Wproj.astype(ml_dtypes.bfloat16)
    bT_host = np.ascontiguousarray(bproj.reshape(1, C)).astype(ml_dtypes.bfloat16)
    band = _band_mask_np(N, W)

    # packed weight layouts (shared by all cores)
    wqkp_host = np.concatenate(
        [np.concatenate(
            [wqk_host[128 * c:128 * (c + 1), 128 * fm:128 * (fm + 1)]
             for c in range(8)] +
            [wqk_host[128 * c:128 * (c + 1), 1024 + 128 * fm:1024 + 128 * (fm + 1)]
             for c in range(8)], axis=1)
         for fm in range(8)], axis=1)
    wqkp_host = np.ascontiguousarray(wqkp_host)
    wvp_host = np.concatenate(
        [np.concatenate([wv_host[128 * c:128 * (c + 1), 512 * half:512 * (half + 1)]
                         for c in range(8)], axis=1)
         for half in range(2)], axis=1)
    wvp_host = np.ascontiguousarray(wvp_host)
    wpp_host = np.concatenate(
        [wp_host[128 * c:128 * (c + 1), :] for c in range(8)], axis=1)
    wpp_host = np.ascontiguousarray(wpp_host)

    in_maps = []
    for core in range(CORES):
        b, qt = divmod(core, 4)
        g0 = qt * TOK
        xhrows = np.zeros((HALO, C), np.float32)
        s = max(0, g0 - W)
        e = min(N, g0 + TOK + W)
        xhrows[s - (g0 - W):e - (g0 - W)] = x[b, s:e]
        xhT_host = np.ascontiguousarray(xhrows.T).astype(ml_dtypes.bfloat16)
        xhp_host = np.ascontiguousarray(np.concatenate(
            [xhT_host[128 * c:128 * (c + 1), :] for c in range(8)], axis=1))

        # mask strip in 112-tile packing: col QT*t + r <-> query g0+QT*t+r,
        # row w <-> key (g0 - W) + QT*t + w
        mAh = np.zeros((128, TOK), np.float32)
        col = 0
        for t, (pq, pw) in enumerate(WTS):
            s0 = QT * t
            i = g0 + s0 + np.arange(pq)[None, :]
            jw = (g0 - W) + s0 + np.arange(pw)[:, None]
            valid = (jw >= 0) & (jw < N)
            mm = band[i, np.clip(jw, 0, N - 1)] & valid
            mAh[0:pw, col:col + pq] = mm
            col += pq
        in_maps.append({
            "xhp": xhp_host, "wvp": wvp_host, "wqkp": wqkp_host,
            "wpp": wpp_host, "bT": bT_host,
            "mA": mAh.astype(ml_dtypes.bfloat16),
        })
    return in_maps


def run_spmd(x, Wqkv, Wproj, bproj, dbg=False, **kw):
    """Run the SPMD kernel; returns (output, BassKernelResults)."""
    nc = _get_nc(dbg)
    in_maps = _make_in_maps(x, Wqkv, Wproj, bproj)
    res = run_bass_kernel_spmd(nc, in_maps, list(range(CORES)), **kw)
    outT = np.concatenate(
        [np.asarray(res.results[i]["outT"], dtype=np.float32)
         for i in range(CORES)], axis=1)
    out = np.ascontiguousarray(outT.T).reshape(B, N, C)
    return out, res


def kernel(x, Wqkv, Wproj, bproj):
    out, _ = run_spmd(x, Wqkv, Wproj, bproj)
    return out
